# revision 27
# baseline (speedup 1.0000x reference)
"""Trainium2 Bass kernel for nn_AttentionAIC (joint text-image attention block).

Sharding: tensor parallel over heads. 24 heads / 8 cores = 3 heads per core.
Each core projects Q/K/V for its 3 heads, runs full-sequence attention, and
computes a partial output projection (contraction over its head slice). The
host sums the 8 partials and adds the output bias.

On-device layout: activations are kept feature-on-partition ("transposed"):
  QnT/KnT: [head_dim, tokens] per head; scores computed as S^T [k, q] so the
  softmax denominator comes from a ones-augmented V in the PV matmul, and no
  transposes are needed in the attention inner loop.
Matmuls run in float32r (full-rate fp32 PE mode, ~1e-4 rel err).

Only the hidden-stream queries are computed (the reference discards the
encoder-stream output), so add_q_proj / norm_added_q / to_add_out are unused.
"""
import sys
import numpy as np
from contextlib import ExitStack

for _p in ("/opt/trn_rl_repo", "/root/.axon_site/_ro/trn_rl_repo"):
    if _p not in sys.path:
        sys.path.append(_p)

S, SE, D, H, HD = 2048, 256, 1536, 24, 64
T = S + SE                 # 2304
NCORES, HPC = 8, 3
J = HPC * HD               # 192 features per core
NKT = D // 128             # 12 contraction tiles for projections
TKT = T // 128             # 18 key tiles
QCH = S // 512             # 4 query chunks of 512
SCALE = float(HD) ** -0.5
EPS = 1e-6

_cached_nc = None


def _build():
    import concourse.bass as bass  # noqa: F401
    import concourse.tile as tile
    from concourse import bacc, mybir

    f32 = mybir.dt.float32
    f32r = mybir.dt.float32r
    AF = mybir.ActivationFunctionType

    nc = bacc.Bacc("TRN2", target_bir_lowering=False, debug=False,
                   num_devices=NCORES)

    def din(name, shape):
        return nc.dram_tensor(name, shape, f32, kind="ExternalInput").ap()

    hsT = din("hsT", [D, S])
    ehsT = din("ehsT", [D, SE])
    maskp = din("maskp", [128, TKT])
    # per-core weight slices, pre-transposed to lhsT layout [contraction, M]
    wkA = din("wkA", [D, 128])     # K proj, heads 0,1
    wqA = din("wqA", [D, 128])     # Q proj, heads 0,1
    wvA = din("wvA", [D, 128])     # V proj, heads 0,1
    wqvB = din("wqvB", [D, 128])   # [Q head2 | V head2] packed
    wkB = din("wkB", [D, 64])      # K head2
    awkA = din("awkA", [D, 128])   # encoder K proj, heads 0,1
    awvA = din("awvA", [D, 128])   # encoder V proj, heads 0,1
    wkvBe = din("wkvBe", [D, 128])  # [encK head2 | encV head2] packed
    bias8 = din("bias8", [256, 5])  # rows 0:128 A, 128:192 B, 192:256 B again
    onesP2 = din("onesP2", [128, 2])
    onesP1 = din("onesP1", [64, 1])
    wqbcA = din("wqbcA", [2, 128])
    wqbcB = din("wqbcB", [1, 64])
    wkbcA = din("wkbcA", [2, 128])
    wkbcB = din("wkbcB", [1, 64])
    wakbcA = din("wakbcA", [2, 128])
    wakbcB = din("wakbcB", [1, 64])
    wo = din("wo", [J, D])
    ident = din("ident", [128, 128])
    yT = nc.dram_tensor("yT", [D, S], f32, kind="ExternalOutput").ap()

    with tile.TileContext(nc) as tc, ExitStack() as top:
        acts = top.enter_context(tc.tile_pool(name="acts", bufs=1))
        const = top.enter_context(tc.tile_pool(name="const", bufs=1))
        vaug = top.enter_context(tc.tile_pool(name="vaug", bufs=TKT))
        va_tiles = [vaug.tile([128, 195], f32r, tag="vaug", name=f"va{tt}")
                    for tt in range(TKT)]
        _one = nc.const_aps.tensor(1.0, (128, 1))
        for _tt in range(TKT):
            for _c in (64, 129, 194):
                nc.gpsimd.tensor_copy(va_tiles[_tt][:, _c:_c + 1], _one)

        qnA = acts.tile([128, S], f32r, tag="qnA")
        qnB = acts.tile([64, S], f32r, tag="qnB")
        knA = acts.tile([128, T], f32r, tag="knA")
        knB = acts.tile([64, T], f32r, tag="knB")
        vtA = acts.tile([128, T], f32, tag="vtA")
        vtB2 = acts.tile([128, T], f32, tag="vtB2")   # head2 V in rows 64:128
        ohA = acts.tile([128, S], f32r, tag="ohA")
        ohB = acts.tile([64, S], f32r, tag="ohB")

        # ---------------- projection phase ----------------
        with ExitStack() as proj:
            wpool = proj.enter_context(tc.tile_pool(name="w", bufs=5))
            hspool = proj.enter_context(tc.tile_pool(name="hs", bufs=16))
            rawpool = proj.enter_context(tc.tile_pool(name="raw", bufs=4))
            sqpool = proj.enter_context(tc.tile_pool(name="sq", bufs=3))
            smpool = proj.enter_context(tc.tile_pool(name="sm", bufs=3))
            pp = proj.enter_context(tc.tile_pool(name="pp", bufs=3, space="PSUM"))
            vbps = proj.enter_context(tc.tile_pool(name="vb", bufs=2, space="PSUM"))
            pst = proj.enter_context(tc.tile_pool(name="pst", bufs=2, space="PSUM"))

            # first-needed loads go first: K/Q weights + first hidden chunk
            def wload(name, d, cols):
                t = wpool.tile([128, NKT, cols], f32r, tag="w", name=name)
                nc.sync.dma_start(
                    t[:], d.bitcast(f32r).rearrange("(a p) m -> p a m", p=128))
                return t

            wkA_t = wload("wkA_t", wkA, 128)
            hs_chunk = {}

            def hsload(k, ch):
                t = hspool.tile([128, 512], f32r, tag="hs", name=f"hs{ch}_{k}")
                nc.sync.dma_start(
                    t[:], hsT.bitcast(f32r)[k * 128:(k + 1) * 128,
                                            ch * 512:(ch + 1) * 512])
                return t

            for k in range(4):
                hs_chunk[k] = hsload(k, 0)
            wqA_t = wload("wqA_t", wqA, 128)
            for k in range(4, 8):
                hs_chunk[k] = hsload(k, 0)
            wvA_t = wload("wvA_t", wvA, 128)
            wqvB_t = wload("wqvB_t", wqvB, 128)
            for k in range(8, NKT):
                hs_chunk[k] = hsload(k, 0)
            wkB_t = wload("wkB_t", wkB, 64)
            hs0 = [hs_chunk[k] for k in range(NKT)]

            # small constants
            maskp_t = const.tile([128, TKT], f32)
            nc.sync.dma_start(maskp_t[:], maskp[:])
            biasA = const.tile([128, 5], f32)
            nc.sync.dma_start(biasA[:], bias8[0:128, :])
            biasB = const.tile([128, 5], f32)
            nc.sync.dma_start(biasB[:], bias8[128:256, :])
            onesP2_t = const.tile([128, 2], f32r)
            nc.sync.dma_start(onesP2_t[:], onesP2.bitcast(f32r))
            onesP1_t = const.tile([64, 1], f32r)
            nc.sync.dma_start(onesP1_t[:], onesP1.bitcast(f32r))
            wbcA_t = {}
            wbcB_t = {}
            for nm, dA, dB in (("q", wqbcA, wqbcB), ("k", wkbcA, wkbcB),
                               ("ak", wakbcA, wakbcB)):
                ta = const.tile([2, 128], f32r, tag=f"wbcA_{nm}")
                nc.sync.dma_start(ta[:], dA.bitcast(f32r))
                tb = const.tile([1, 64], f32r, tag=f"wbcB_{nm}")
                nc.sync.dma_start(tb[:], dB.bitcast(f32r))
                wbcA_t[nm] = ta
                wbcB_t[nm] = tb
            ident_t = const.tile([128, 128], f32)
            nc.sync.dma_start(ident_t[:], ident[:])
            eps_t = const.tile([128, 1], f32)
            nc.vector.memset(eps_t[:], EPS)

            def norm_chain(ps_ap, bias_ap, raw_t, P, W, ones_t, wbc_t,
                           dst_ap, uid):
                """dst = raw * rsqrt(mean(raw^2, per 64-row head) + eps) * w"""
                sq = sqpool.tile([P, W], f32r, tag="sq", name=f"sq{uid}")
                nc.scalar.activation(sq[:], ps_ap, AF.Square, bias=bias_ap)
                nh = 2 if P == 128 else 1
                var = vbps.tile([nh, W], f32, tag="vb", name=f"var{uid}")
                nc.tensor.matmul(var[:], ones_t[:], sq[:],
                                 start=True, stop=True)
                sd = smpool.tile([nh, W], f32, tag="sd", name=f"sd{uid}")
                nc.scalar.activation(sd[:], var[:], AF.Sqrt,
                                     bias=eps_t[0:nh, 0:1], scale=1.0 / HD)
                rinv = smpool.tile([nh, W], f32r, tag="rinv", name=f"ri{uid}")
                with nc.allow_low_precision(reason="f32r matmul operand"):
                    nc.vector.reciprocal(rinv[:], sd[:])
                bc = vbps.tile([P, W], f32, tag="vb", name=f"bc{uid}")
                nc.tensor.matmul(bc[:], wbc_t[:], rinv[:],
                                 start=True, stop=True)
                nc.vector.tensor_mul(dst_ap, raw_t[:], bc[:])

            def kproj(wt, rhs, W, P, uid):
                """Run one 12-deep accumulation pass; returns the psum tile."""
                ps = pp.tile([P, W], f32, tag="pp", name=f"ps{uid}")
                for k in range(NKT):
                    nc.tensor.matmul(ps[:], wt[:, k, :], rhs[k][:],
                                     start=(k == 0), stop=(k == NKT - 1))
                return ps

            def evac(dst_ap, ps_ap, bias_ap, uid, engine="v"):
                nc.vector.tensor_scalar_add(dst_ap, ps_ap, bias_ap)

            def raw_evac(ps_ap, P, W, bias_ap, uid):
                raw = rawpool.tile([P, W], f32, tag="raw", name=f"raw{uid}")
                nc.vector.tensor_scalar_add(raw[:], ps_ap, bias_ap)
                return raw

            def transposes_A_proj(tt):
                tsl = slice(tt * 128, (tt + 1) * 128)
                va = va_tiles[tt]
                tp = pst.tile([128, 128], f32, tag="tp", name=f"tp{tt}")
                nc.tensor.transpose(tp[:], vtA[:, tsl], ident_t[:])
                nc.vector.tensor_copy(va[:, 0:64], tp[:, 0:64])
                nc.vector.tensor_copy(va[:, 65:129], tp[:, 64:128])

            def transposes_B(tt):
                tsl = slice(tt * 128, (tt + 1) * 128)
                va = va_tiles[tt]
                tp2 = pst.tile([128, 64], f32, tag="tp", name=f"tq{tt}")
                nc.tensor.matmul(tp2[:], vtB2[64:128, tsl],
                                 ident_t[64:128, 64:128], is_transpose=True,
                                 start=True, stop=True, tile_position=(64, 0))
                nc.vector.tensor_copy(va[:, 130:194], tp2[:, 0:64])

            for ch in range(QCH):
                if ch == 0:
                    hs_tiles = hs0
                else:
                    hs_tiles = [hsload(k, ch) for k in range(NKT)]
                csl = slice(ch * 512, (ch + 1) * 512)
                u = f"h{ch}"
                # K heads 0,1
                ps = kproj(wkA_t, hs_tiles, 512, 128, u + "kA")
                raw = raw_evac(ps[:], 128, 512, biasA[:, 1:2], u + "kA")
                norm_chain(ps[:], biasA[:, 1:2], raw, 128, 512, onesP2_t,
                           wbcA_t["k"], knA[:, csl], u + "kA")
                # Q heads 0,1
                ps = kproj(wqA_t, hs_tiles, 512, 128, u + "qA")
                raw = raw_evac(ps[:], 128, 512, biasA[:, 0:1], u + "qA")
                norm_chain(ps[:], biasA[:, 0:1], raw, 128, 512, onesP2_t,
                           wbcA_t["q"], qnA[:, csl], u + "qA")
                ps = kproj(wvA_t, hs_tiles, 512, 128, u + "vA")
                evac(vtA[:, csl], ps[:], biasA[:, 2:3], u + "vA")
                # K head 2
                ps = kproj(wkB_t, hs_tiles, 512, 64, u + "kB")
                raw = raw_evac(ps[:], 64, 512, biasB[0:64, 1:2], u + "kB")
                norm_chain(ps[:], biasB[0:64, 1:2], raw, 64, 512, onesP1_t,
                           wbcB_t["k"], knB[:, csl], u + "kB")
                # packed [Q head2 | V head2]
                ps = kproj(wqvB_t, hs_tiles, 512, 128, u + "qvB")
                raw = raw_evac(ps[0:64, :], 64, 512, biasB[0:64, 0:1], u + "qB")
                norm_chain(ps[0:64, :], biasB[0:64, 0:1], raw, 64, 512,
                           onesP1_t, wbcB_t["q"], qnB[:, csl], u + "qB")
                evac(vtB2[64:128, csl], ps[64:128, :], biasB[64:128, 2:3],
                     u + "vB")
                for tt in range(ch * 4, ch * 4 + 4):
                    transposes_B(tt)
                    transposes_A_proj(tt)

            # ---------------- encoder stream (keys/values only) --------------
            ehs_t = acts.tile([128, NKT, SE], f32r, tag="ehs")
            nc.sync.dma_start(ehs_t[:],
                              ehsT.bitcast(f32r).rearrange("(a p) m -> p a m",
                                                           p=128))
            awkA_t = wload("awkA_t", awkA, 128)
            awvA_t = wload("awvA_t", awvA, 128)
            wkvBe_t = wload("wkvBe_t", wkvBe, 128)

            class _W:  # wrap a pre-sliced AP so kproj can do [k][:]
                def __init__(self, ap):
                    self.ap = ap

                def __getitem__(self, s):
                    return self.ap
            ehs_w = [_W(ehs_t[:, k, :]) for k in range(NKT)]
            esl = slice(S, T)

            ps = kproj(awkA_t, ehs_w, SE, 128, "ekA")
            raw = raw_evac(ps[:], 128, SE, biasA[:, 3:4], "ekA")
            norm_chain(ps[:], biasA[:, 3:4], raw, 128, SE, onesP2_t,
                       wbcA_t["ak"], knA[:, esl], "ekA")
            ps = kproj(awvA_t, ehs_w, SE, 128, "evA")
            evac(vtA[:, esl], ps[:], biasA[:, 4:5], "evA")
            ps = kproj(wkvBe_t, ehs_w, SE, 128, "ekvB")
            raw = raw_evac(ps[0:64, :], 64, SE, biasB[0:64, 3:4], "ekB")
            norm_chain(ps[0:64, :], biasB[0:64, 3:4], raw, 64, SE, onesP1_t,
                       wbcB_t["ak"], knB[:, esl], "ekB")
            evac(vtB2[64:128, esl], ps[64:128, :], biasB[64:128, 4:5], "evB")
            for tt in (16, 17):
                transposes_B(tt)
                transposes_A_proj(tt)

        # ---------------- attention + output projection ----------------
        with ExitStack() as att:
            scps = att.enter_context(tc.tile_pool(name="sc", bufs=2, space="PSUM"))
            pvps = att.enter_context(tc.tile_pool(name="pv", bufs=2, space="PSUM"))
            yps = att.enter_context(tc.tile_pool(name="yp", bufs=2, space="PSUM"))
            epool = att.enter_context(tc.tile_pool(name="e", bufs=4))
            bcsb = att.enter_context(tc.tile_pool(name="bcs", bufs=2))
            smp2 = att.enter_context(tc.tile_pool(name="sm2", bufs=2))
            yev = att.enter_context(tc.tile_pool(name="ye", bufs=3))

            woA_t = acts.tile([128, D], f32r, tag="woA")
            nc.sync.dma_start(woA_t[:], wo.bitcast(f32r)[0:128, :])
            woB_t = acts.tile([64, D], f32r, tag="woB")
            nc.sync.dma_start(woB_t[:], wo.bitcast(f32r)[128:192, :])

            for qh in range(2):
                for h in range(HPC):
                    if h < 2:
                        kn_t, qn_t, oh_t = knA, qnA, ohA
                        base = 64 * h
                        tpos = (base, 0)
                    else:
                        kn_t, qn_t, oh_t = knB, qnB, ohB
                        base = 0
                        tpos = (0, 0)
                    pv = [pvps.tile([65, 512], f32, tag="pv",
                                    name=f"pv{h}_{qh}_{i}") for i in range(2)]
                    for kt in range(TKT):
                        sc = scps.tile([128, 1024], f32, tag="sc",
                                       name=f"sc{h}_{qh}_{kt}")
                        for s2 in range(2):
                            q0 = qh * 1024 + s2 * 512
                            nc.tensor.matmul(
                                sc[:, s2 * 512:(s2 + 1) * 512],
                                kn_t[base:base + 64,
                                     kt * 128:(kt + 1) * 128],
                                qn_t[base:base + 64, q0:q0 + 512],
                                start=True, stop=True, tile_position=tpos)
                        e = epool.tile([128, 1024], f32r, tag="e",
                                       name=f"e{h}_{qh}_{kt}")
                        nc.scalar.activation(e[:], sc[:], AF.Exp,
                                             bias=maskp_t[:, kt:kt + 1],
                                             scale=SCALE)
                        for s2 in range(2):
                            nc.tensor.matmul(
                                pv[s2][:],
                                va_tiles[kt][:, 65 * h:65 * h + 65],
                                e[:, s2 * 512:(s2 + 1) * 512],
                                start=(kt == 0), stop=(kt == TKT - 1))
                    for s2 in range(2):
                        q0 = qh * 1024 + s2 * 512
                        rinv = smp2.tile([1, 512], f32, tag="rinvu",
                                         name=f"ri{h}_{qh}_{s2}")
                        nc.vector.reciprocal(rinv[:], pv[s2][64:65, :])
                        bc = bcsb.tile([64, 512], f32, tag="bcs",
                                       name=f"bc{h}_{qh}_{s2}")
                        nc.gpsimd.partition_broadcast(bc[:], rinv[:])
                        nc.vector.tensor_mul(oh_t[base:base + 64, q0:q0 + 512],
                                             pv[s2][0:64, :], bc[:])
                # partial output projection for this query half
                for od in range(NKT):
                    osl = slice(od * 128, (od + 1) * 128)
                    ys = yev.tile([128, 1024], f32, tag="ye",
                                  name=f"ye{qh}_{od}")
                    for qt in range(2):
                        qsl = slice(qh * 1024 + qt * 512,
                                    qh * 1024 + (qt + 1) * 512)
                        yp = yps.tile([128, 512], f32, tag="yp",
                                      name=f"yp{qh}_{od}_{qt}")
                        nc.tensor.matmul(yp[:], woA_t[:, osl], ohA[:, qsl],
                                         start=True, stop=False)
                        nc.tensor.matmul(yp[:], woB_t[:, osl], ohB[:, qsl],
                                         start=False, stop=True)
                        dst = ys[:, qt * 512:(qt + 1) * 512]
                        if qh == 1 and qt == 1:
                            nc.scalar.copy(dst, yp[:])
                        else:
                            nc.vector.tensor_copy(dst, yp[:])
                    nc.sync.dma_start(
                        yT[osl, qh * 1024:(qh + 1) * 1024], ys[:])

    nc.compile()
    return nc


def _prep_inputs(inputs):
    g = {k: np.asarray(v) for k, v in inputs.items()}
    hs = g["hidden_states"].astype(np.float32).reshape(S, D)
    ehs = g["encoder_hidden_states"].astype(np.float32).reshape(SE, D)
    mask = g["attention_mask"].astype(np.float32).reshape(-1)
    assert mask.shape[0] == T

    shared = {
        "hsT": np.ascontiguousarray(hs.T),
        "ehsT": np.ascontiguousarray(ehs.T),
        "maskp": np.ascontiguousarray(mask.reshape(TKT, 128).T),
        "ident": np.eye(128, dtype=np.float32),
    }
    onesP2 = np.zeros((128, 2), np.float32)
    onesP2[0:64, 0] = 1.0
    onesP2[64:128, 1] = 1.0
    shared["onesP2"] = onesP2
    shared["onesP1"] = np.ones((64, 1), np.float32)

    def blocks(w):
        a = np.zeros((2, 128), np.float32)
        a[0, 0:64] = w
        a[1, 64:128] = w
        return a, np.ascontiguousarray(w.reshape(1, 64))

    for nm, key in (("wq", "norm_q_weight"), ("wk", "norm_k_weight"),
                    ("wak", "norm_added_k_weight")):
        a, b = blocks(g[key].astype(np.float32))
        shared[nm + "bcA"] = a
        shared[nm + "bcB"] = b

    f32 = np.float32
    in_maps = []
    for c in range(NCORES):
        sl = slice(c * J, (c + 1) * J)
        wq = g["to_q_weight"].astype(f32)[sl, :].T    # [D, 192]
        wk = g["to_k_weight"].astype(f32)[sl, :].T
        wv = g["to_v_weight"].astype(f32)[sl, :].T
        awk = g["add_k_proj_weight"].astype(f32)[sl, :].T
        awv = g["add_v_proj_weight"].astype(f32)[sl, :].T
        m = dict(shared)
        m["wkA"] = np.ascontiguousarray(wk[:, 0:128])
        m["wqA"] = np.ascontiguousarray(wq[:, 0:128])
        m["wvA"] = np.ascontiguousarray(wv[:, 0:128])
        m["wqvB"] = np.ascontiguousarray(
            np.concatenate([wq[:, 128:192], wv[:, 128:192]], axis=1))
        m["wkB"] = np.ascontiguousarray(wk[:, 128:192])
        m["awkA"] = np.ascontiguousarray(awk[:, 0:128])
        m["awvA"] = np.ascontiguousarray(awv[:, 0:128])
        m["wkvBe"] = np.ascontiguousarray(
            np.concatenate([awk[:, 128:192], awv[:, 128:192]], axis=1))
        m["wo"] = np.ascontiguousarray(g["to_out_0_weight"].astype(f32)[:, sl].T)
        b5 = np.stack(
            [g["to_q_bias"].astype(f32)[sl],
             g["to_k_bias"].astype(f32)[sl],
             g["to_v_bias"].astype(f32)[sl],
             g["add_k_proj_bias"].astype(f32)[sl],
             g["add_v_proj_bias"].astype(f32)[sl]], axis=1)   # [192, 5]
        m["bias8"] = np.ascontiguousarray(
            np.concatenate([b5, b5[128:192, :]], axis=0))     # [256, 5]
        in_maps.append(m)
    return in_maps


def kernel(**inputs):
    global _cached_nc
    from concourse import bass_utils

    if _cached_nc is None:
        _cached_nc = _build()
    nc = _cached_nc

    in_maps = _prep_inputs(inputs)
    res = bass_utils.run_bass_kernel_spmd(nc, in_maps,
                                          core_ids=list(range(NCORES)))
    acc = res.results[0]["yT"].copy()
    for c in range(1, NCORES):
        acc += res.results[c]["yT"]
    bias = np.asarray(inputs["to_out_0_bias"]).astype(np.float32)
    out = acc.T + bias[None, :]
    return out.reshape(1, S, D).astype(np.float32)


if __name__ == "__main__":
    rng = np.random.default_rng(0)
    ins = {
        "hidden_states": rng.standard_normal((1, S, D)).astype(np.float32),
        "encoder_hidden_states": rng.standard_normal((1, SE, D)).astype(np.float32),
        "attention_mask": np.zeros((1, 1, 1, T), np.float32),
        "to_q_weight": (rng.standard_normal((D, D)) * 0.02).astype(np.float32),
        "to_q_bias": np.zeros(D, np.float32),
        "to_k_weight": (rng.standard_normal((D, D)) * 0.02).astype(np.float32),
        "to_k_bias": np.zeros(D, np.float32),
        "to_v_weight": (rng.standard_normal((D, D)) * 0.02).astype(np.float32),
        "to_v_bias": np.zeros(D, np.float32),
        "norm_q_weight": np.ones(HD, np.float32),
        "norm_k_weight": np.ones(HD, np.float32),
        "add_q_proj_weight": (rng.standard_normal((D, D)) * 0.02).astype(np.float32),
        "add_q_proj_bias": np.zeros(D, np.float32),
        "add_k_proj_weight": (rng.standard_normal((D, D)) * 0.02).astype(np.float32),
        "add_k_proj_bias": np.zeros(D, np.float32),
        "add_v_proj_weight": (rng.standard_normal((D, D)) * 0.02).astype(np.float32),
        "add_v_proj_bias": np.zeros(D, np.float32),
        "norm_added_q_weight": np.ones(HD, np.float32),
        "norm_added_k_weight": np.ones(HD, np.float32),
        "to_out_0_weight": (rng.standard_normal((D, D)) * 0.02).astype(np.float32),
        "to_out_0_bias": np.zeros(D, np.float32),
        "to_add_out_weight": (rng.standard_normal((D, D)) * 0.02).astype(np.float32),
        "to_add_out_bias": np.zeros(D, np.float32),
        "attn_heads": 24,
        "attn_head_dim": 64,
    }
    out = kernel(**ins)
    print("out", out.shape, out.dtype, float(np.abs(out).max()))


# revision 32
# speedup vs baseline: 1.0012x; 1.0012x over previous
"""Trainium2 Bass kernel for nn_AttentionAIC (joint text-image attention block).

Sharding: tensor parallel over heads. 24 heads / 8 cores = 3 heads per core.
Each core projects Q/K/V for its 3 heads, runs full-sequence attention, and
computes a partial output projection (contraction over its head slice). The
host sums the 8 partials and adds the output bias.

On-device layout: activations are kept feature-on-partition ("transposed"):
  QnT/KnT: [head_dim, tokens] per head; scores computed as S^T [k, q] so the
  softmax denominator comes from a ones-augmented V in the PV matmul, and no
  transposes are needed in the attention inner loop.
Matmuls run in float32r (full-rate fp32 PE mode, ~1e-4 rel err).

Only the hidden-stream queries are computed (the reference discards the
encoder-stream output), so add_q_proj / norm_added_q / to_add_out are unused.
"""
import sys
import numpy as np
from contextlib import ExitStack

for _p in ("/opt/trn_rl_repo", "/root/.axon_site/_ro/trn_rl_repo"):
    if _p not in sys.path:
        sys.path.append(_p)

S, SE, D, H, HD = 2048, 256, 1536, 24, 64
T = S + SE                 # 2304
NCORES, HPC = 8, 3
J = HPC * HD               # 192 features per core
NKT = D // 128             # 12 contraction tiles for projections
TKT = T // 128             # 18 key tiles
QCH = S // 512             # 4 query chunks of 512
SCALE = float(HD) ** -0.5
EPS = 1e-6

_cached_nc = None


def _build():
    import concourse.bass as bass  # noqa: F401
    import concourse.tile as tile
    from concourse import bacc, mybir

    f32 = mybir.dt.float32
    f32r = mybir.dt.float32r
    AF = mybir.ActivationFunctionType

    nc = bacc.Bacc("TRN2", target_bir_lowering=False, debug=False,
                   num_devices=NCORES)

    def din(name, shape):
        return nc.dram_tensor(name, shape, f32, kind="ExternalInput").ap()

    hsT = din("hsT", [D, S])
    ehsT = din("ehsT", [D, SE])
    maskp = din("maskp", [128, TKT])
    # per-core weight slices, pre-transposed to lhsT layout [contraction, M]
    wkA = din("wkA", [D, 128])     # K proj, heads 0,1
    wqA = din("wqA", [D, 128])     # Q proj, heads 0,1
    wvA = din("wvA", [D, 128])     # V proj, heads 0,1
    wqvB = din("wqvB", [D, 128])   # [Q head2 | V head2] packed
    wkB = din("wkB", [D, 64])      # K head2
    awkA = din("awkA", [D, 128])   # encoder K proj, heads 0,1
    awvA = din("awvA", [D, 128])   # encoder V proj, heads 0,1
    wkvBe = din("wkvBe", [D, 128])  # [encK head2 | encV head2] packed
    bias8 = din("bias8", [256, 5])  # rows 0:128 A, 128:192 B, 192:256 B again
    onesP2 = din("onesP2", [128, 2])
    onesP1 = din("onesP1", [64, 1])
    wqbcA = din("wqbcA", [2, 128])
    wqbcB = din("wqbcB", [1, 64])
    wkbcA = din("wkbcA", [2, 128])
    wkbcB = din("wkbcB", [1, 64])
    wakbcA = din("wakbcA", [2, 128])
    wakbcB = din("wakbcB", [1, 64])
    wo = din("wo", [J, D])
    ident = din("ident", [128, 128])
    yT = nc.dram_tensor("yT", [D, S], f32, kind="ExternalOutput").ap()

    with tile.TileContext(nc) as tc, ExitStack() as top:
        acts = top.enter_context(tc.tile_pool(name="acts", bufs=1))
        const = top.enter_context(tc.tile_pool(name="const", bufs=1))
        vaug = top.enter_context(tc.tile_pool(name="vaug", bufs=TKT))
        va_tiles = [vaug.tile([128, 195], f32r, tag="vaug", name=f"va{tt}")
                    for tt in range(TKT)]
        _one = nc.const_aps.tensor(1.0, (128, 1))
        for _tt in range(TKT):
            for _c in (64, 129, 194):
                nc.gpsimd.tensor_copy(va_tiles[_tt][:, _c:_c + 1], _one)

        qnA = acts.tile([128, S], f32r, tag="qnA")
        qnB = acts.tile([64, S], f32r, tag="qnB")
        knA = acts.tile([128, T], f32r, tag="knA")
        knB = acts.tile([64, T], f32r, tag="knB")
        vtA = acts.tile([128, T], f32, tag="vtA")
        vtB2 = acts.tile([128, T], f32, tag="vtB2")   # head2 V in rows 64:128
        ohA = acts.tile([128, S], f32r, tag="ohA")
        ohB = acts.tile([64, S], f32r, tag="ohB")

        # ---------------- projection phase ----------------
        with ExitStack() as proj:
            wpool = proj.enter_context(tc.tile_pool(name="w", bufs=5))
            hspool = proj.enter_context(tc.tile_pool(name="hs", bufs=16))
            rawpool = proj.enter_context(tc.tile_pool(name="raw", bufs=6))
            sqpool = proj.enter_context(tc.tile_pool(name="sq", bufs=4))
            smpool = proj.enter_context(tc.tile_pool(name="sm", bufs=3))
            pp = proj.enter_context(tc.tile_pool(name="pp", bufs=3, space="PSUM"))
            vbps = proj.enter_context(tc.tile_pool(name="vb", bufs=3, space="PSUM"))
            pst = proj.enter_context(tc.tile_pool(name="pst", bufs=2, space="PSUM"))

            # first-needed loads go first: K/Q weights + first hidden chunk
            def wload(name, d, cols):
                t = wpool.tile([128, NKT, cols], f32r, tag="w", name=name)
                nc.sync.dma_start(
                    t[:], d.bitcast(f32r).rearrange("(a p) m -> p a m", p=128))
                return t

            wkA_t = wload("wkA_t", wkA, 128)
            hs_chunk = {}

            def hsload(k, ch):
                t = hspool.tile([128, 512], f32r, tag="hs", name=f"hs{ch}_{k}")
                nc.sync.dma_start(
                    t[:], hsT.bitcast(f32r)[k * 128:(k + 1) * 128,
                                            ch * 512:(ch + 1) * 512])
                return t

            for k in range(4):
                hs_chunk[k] = hsload(k, 0)
            wqA_t = wload("wqA_t", wqA, 128)
            for k in range(4, 8):
                hs_chunk[k] = hsload(k, 0)
            wvA_t = wload("wvA_t", wvA, 128)
            wqvB_t = wload("wqvB_t", wqvB, 128)
            for k in range(8, NKT):
                hs_chunk[k] = hsload(k, 0)
            wkB_t = wload("wkB_t", wkB, 64)
            hs0 = [hs_chunk[k] for k in range(NKT)]

            # small constants
            maskp_t = const.tile([128, TKT], f32)
            nc.sync.dma_start(maskp_t[:], maskp[:])
            biasA = const.tile([128, 5], f32)
            nc.sync.dma_start(biasA[:], bias8[0:128, :])
            biasB = const.tile([128, 5], f32)
            nc.sync.dma_start(biasB[:], bias8[128:256, :])
            onesP2_t = const.tile([128, 2], f32r)
            nc.sync.dma_start(onesP2_t[:], onesP2.bitcast(f32r))
            onesP1_t = const.tile([64, 1], f32r)
            nc.sync.dma_start(onesP1_t[:], onesP1.bitcast(f32r))
            wbcA_t = {}
            wbcB_t = {}
            for nm, dA, dB in (("q", wqbcA, wqbcB), ("k", wkbcA, wkbcB),
                               ("ak", wakbcA, wakbcB)):
                ta = const.tile([2, 128], f32r, tag=f"wbcA_{nm}")
                nc.sync.dma_start(ta[:], dA.bitcast(f32r))
                tb = const.tile([1, 64], f32r, tag=f"wbcB_{nm}")
                nc.sync.dma_start(tb[:], dB.bitcast(f32r))
                wbcA_t[nm] = ta
                wbcB_t[nm] = tb
            ident_t = const.tile([128, 128], f32)
            nc.sync.dma_start(ident_t[:], ident[:])
            eps_t = const.tile([128, 1], f32)
            nc.vector.memset(eps_t[:], EPS)

            def norm_chain(ps_ap, bias_ap, raw_t, P, W, ones_t, wbc_t,
                           dst_ap, uid):
                """dst = raw * rsqrt(mean(raw^2, per 64-row head) + eps) * w"""
                sq = sqpool.tile([P, W], f32r, tag="sq", name=f"sq{uid}")
                nc.scalar.activation(sq[:], ps_ap, AF.Square, bias=bias_ap)
                nh = 2 if P == 128 else 1
                var = vbps.tile([nh, W], f32, tag="vb", name=f"var{uid}")
                nc.tensor.matmul(var[:], ones_t[:], sq[:],
                                 start=True, stop=True)
                sd = smpool.tile([nh, W], f32, tag="sd", name=f"sd{uid}")
                nc.scalar.activation(sd[:], var[:], AF.Sqrt,
                                     bias=eps_t[0:nh, 0:1], scale=1.0 / HD)
                rinv = smpool.tile([nh, W], f32r, tag="rinv", name=f"ri{uid}")
                with nc.allow_low_precision(reason="f32r matmul operand"):
                    nc.vector.reciprocal(rinv[:], sd[:])
                bc = vbps.tile([P, W], f32, tag="vb", name=f"bc{uid}")
                nc.tensor.matmul(bc[:], wbc_t[:], rinv[:],
                                 start=True, stop=True)
                nc.vector.tensor_mul(dst_ap, raw_t[:], bc[:])

            def kproj(wt, rhs, W, P, uid):
                """Run one 12-deep accumulation pass; returns the psum tile."""
                ps = pp.tile([P, W], f32, tag="pp", name=f"ps{uid}")
                for k in range(NKT):
                    nc.tensor.matmul(ps[:], wt[:, k, :], rhs[k][:],
                                     start=(k == 0), stop=(k == NKT - 1))
                return ps

            def evac(dst_ap, ps_ap, bias_ap, uid, engine="v"):
                nc.vector.tensor_scalar_add(dst_ap, ps_ap, bias_ap)

            def raw_evac(ps_ap, P, W, bias_ap, uid):
                raw = rawpool.tile([P, W], f32, tag="raw", name=f"raw{uid}")
                nc.vector.tensor_scalar_add(raw[:], ps_ap, bias_ap)
                return raw

            def transposes_A_proj(tt):
                tsl = slice(tt * 128, (tt + 1) * 128)
                va = va_tiles[tt]
                tp = pst.tile([128, 128], f32, tag="tp", name=f"tp{tt}")
                nc.tensor.transpose(tp[:], vtA[:, tsl], ident_t[:])
                nc.vector.tensor_copy(va[:, 0:64], tp[:, 0:64])
                nc.vector.tensor_copy(va[:, 65:129], tp[:, 64:128])

            def transposes_B(tt):
                tsl = slice(tt * 128, (tt + 1) * 128)
                va = va_tiles[tt]
                tp2 = pst.tile([128, 64], f32, tag="tp", name=f"tq{tt}")
                nc.tensor.matmul(tp2[:], vtB2[64:128, tsl],
                                 ident_t[64:128, 64:128], is_transpose=True,
                                 start=True, stop=True, tile_position=(64, 0))
                nc.vector.tensor_copy(va[:, 130:194], tp2[:, 0:64])

            for ch in range(QCH):
                if ch == 0:
                    hs_tiles = hs0
                else:
                    hs_tiles = [hsload(k, ch) for k in range(NKT)]
                csl = slice(ch * 512, (ch + 1) * 512)
                u = f"h{ch}"
                # K heads 0,1
                ps = kproj(wkA_t, hs_tiles, 512, 128, u + "kA")
                raw = raw_evac(ps[:], 128, 512, biasA[:, 1:2], u + "kA")
                norm_chain(ps[:], biasA[:, 1:2], raw, 128, 512, onesP2_t,
                           wbcA_t["k"], knA[:, csl], u + "kA")
                # Q heads 0,1
                ps = kproj(wqA_t, hs_tiles, 512, 128, u + "qA")
                raw = raw_evac(ps[:], 128, 512, biasA[:, 0:1], u + "qA")
                norm_chain(ps[:], biasA[:, 0:1], raw, 128, 512, onesP2_t,
                           wbcA_t["q"], qnA[:, csl], u + "qA")
                ps = kproj(wvA_t, hs_tiles, 512, 128, u + "vA")
                evac(vtA[:, csl], ps[:], biasA[:, 2:3], u + "vA")
                # K head 2
                ps = kproj(wkB_t, hs_tiles, 512, 64, u + "kB")
                raw = raw_evac(ps[:], 64, 512, biasB[0:64, 1:2], u + "kB")
                norm_chain(ps[:], biasB[0:64, 1:2], raw, 64, 512, onesP1_t,
                           wbcB_t["k"], knB[:, csl], u + "kB")
                # packed [Q head2 | V head2]
                ps = kproj(wqvB_t, hs_tiles, 512, 128, u + "qvB")
                raw = raw_evac(ps[0:64, :], 64, 512, biasB[0:64, 0:1], u + "qB")
                norm_chain(ps[0:64, :], biasB[0:64, 0:1], raw, 64, 512,
                           onesP1_t, wbcB_t["q"], qnB[:, csl], u + "qB")
                evac(vtB2[64:128, csl], ps[64:128, :], biasB[64:128, 2:3],
                     u + "vB")
                for tt in range(ch * 4, ch * 4 + 4):
                    transposes_B(tt)
                    transposes_A_proj(tt)

            # ---------------- encoder stream (keys/values only) --------------
            ehs_t = acts.tile([128, NKT, SE], f32r, tag="ehs")
            nc.sync.dma_start(ehs_t[:],
                              ehsT.bitcast(f32r).rearrange("(a p) m -> p a m",
                                                           p=128))
            awkA_t = wload("awkA_t", awkA, 128)
            awvA_t = wload("awvA_t", awvA, 128)
            wkvBe_t = wload("wkvBe_t", wkvBe, 128)

            class _W:  # wrap a pre-sliced AP so kproj can do [k][:]
                def __init__(self, ap):
                    self.ap = ap

                def __getitem__(self, s):
                    return self.ap
            ehs_w = [_W(ehs_t[:, k, :]) for k in range(NKT)]
            esl = slice(S, T)

            ps = kproj(awkA_t, ehs_w, SE, 128, "ekA")
            raw = raw_evac(ps[:], 128, SE, biasA[:, 3:4], "ekA")
            norm_chain(ps[:], biasA[:, 3:4], raw, 128, SE, onesP2_t,
                       wbcA_t["ak"], knA[:, esl], "ekA")
            ps = kproj(awvA_t, ehs_w, SE, 128, "evA")
            evac(vtA[:, esl], ps[:], biasA[:, 4:5], "evA")
            ps = kproj(wkvBe_t, ehs_w, SE, 128, "ekvB")
            raw = raw_evac(ps[0:64, :], 64, SE, biasB[0:64, 3:4], "ekB")
            norm_chain(ps[0:64, :], biasB[0:64, 3:4], raw, 64, SE, onesP1_t,
                       wbcB_t["ak"], knB[:, esl], "ekB")
            evac(vtB2[64:128, esl], ps[64:128, :], biasB[64:128, 4:5], "evB")
            for tt in (16, 17):
                transposes_B(tt)
                transposes_A_proj(tt)

        # ---------------- attention + output projection ----------------
        with ExitStack() as att:
            scps = att.enter_context(tc.tile_pool(name="sc", bufs=2, space="PSUM"))
            pvps = att.enter_context(tc.tile_pool(name="pv", bufs=2, space="PSUM"))
            yps = att.enter_context(tc.tile_pool(name="yp", bufs=2, space="PSUM"))
            epool = att.enter_context(tc.tile_pool(name="e", bufs=6))
            bcsb = att.enter_context(tc.tile_pool(name="bcs", bufs=2))
            smp2 = att.enter_context(tc.tile_pool(name="sm2", bufs=2))
            yev = att.enter_context(tc.tile_pool(name="ye", bufs=3))

            woA_t = acts.tile([128, D], f32r, tag="woA")
            nc.sync.dma_start(woA_t[:], wo.bitcast(f32r)[0:128, :])
            woB_t = acts.tile([64, D], f32r, tag="woB")
            nc.sync.dma_start(woB_t[:], wo.bitcast(f32r)[128:192, :])

            for qh in range(2):
                for h in range(HPC):
                    if h < 2:
                        kn_t, qn_t, oh_t = knA, qnA, ohA
                        base = 64 * h
                        tpos = (base, 0)
                    else:
                        kn_t, qn_t, oh_t = knB, qnB, ohB
                        base = 0
                        tpos = (0, 0)
                    pv = [pvps.tile([65, 512], f32, tag="pv",
                                    name=f"pv{h}_{qh}_{i}") for i in range(2)]
                    for kt in range(TKT):
                        sc = scps.tile([128, 1024], f32, tag="sc",
                                       name=f"sc{h}_{qh}_{kt}")
                        for s2 in range(2):
                            q0 = qh * 1024 + s2 * 512
                            nc.tensor.matmul(
                                sc[:, s2 * 512:(s2 + 1) * 512],
                                kn_t[base:base + 64,
                                     kt * 128:(kt + 1) * 128],
                                qn_t[base:base + 64, q0:q0 + 512],
                                start=True, stop=True, tile_position=tpos)
                        e = epool.tile([128, 1024], f32r, tag="e",
                                       name=f"e{h}_{qh}_{kt}")
                        nc.scalar.activation(e[:], sc[:], AF.Exp,
                                             bias=maskp_t[:, kt:kt + 1],
                                             scale=SCALE)
                        for s2 in range(2):
                            nc.tensor.matmul(
                                pv[s2][:],
                                va_tiles[kt][:, 65 * h:65 * h + 65],
                                e[:, s2 * 512:(s2 + 1) * 512],
                                start=(kt == 0), stop=(kt == TKT - 1))
                    for s2 in range(2):
                        q0 = qh * 1024 + s2 * 512
                        rinv = smp2.tile([1, 512], f32, tag="rinvu",
                                         name=f"ri{h}_{qh}_{s2}")
                        nc.vector.reciprocal(rinv[:], pv[s2][64:65, :])
                        bc = bcsb.tile([64, 512], f32, tag="bcs",
                                       name=f"bc{h}_{qh}_{s2}")
                        nc.gpsimd.partition_broadcast(bc[:], rinv[:])
                        nc.vector.tensor_mul(oh_t[base:base + 64, q0:q0 + 512],
                                             pv[s2][0:64, :], bc[:])
                # partial output projection for this query half
                for od in range(NKT):
                    osl = slice(od * 128, (od + 1) * 128)
                    ys = yev.tile([128, 1024], f32, tag="ye",
                                  name=f"ye{qh}_{od}")
                    for qt in range(2):
                        qsl = slice(qh * 1024 + qt * 512,
                                    qh * 1024 + (qt + 1) * 512)
                        yp = yps.tile([128, 512], f32, tag="yp",
                                      name=f"yp{qh}_{od}_{qt}")
                        nc.tensor.matmul(yp[:], woA_t[:, osl], ohA[:, qsl],
                                         start=True, stop=False)
                        nc.tensor.matmul(yp[:], woB_t[:, osl], ohB[:, qsl],
                                         start=False, stop=True)
                        dst = ys[:, qt * 512:(qt + 1) * 512]
                        if qh == 1 and qt == 1:
                            nc.scalar.copy(dst, yp[:])
                        else:
                            nc.vector.tensor_copy(dst, yp[:])
                    nc.sync.dma_start(
                        yT[osl, qh * 1024:(qh + 1) * 1024], ys[:])

    nc.compile()
    return nc


def _prep_inputs(inputs):
    g = {k: np.asarray(v) for k, v in inputs.items()}
    hs = g["hidden_states"].astype(np.float32).reshape(S, D)
    ehs = g["encoder_hidden_states"].astype(np.float32).reshape(SE, D)
    mask = g["attention_mask"].astype(np.float32).reshape(-1)
    assert mask.shape[0] == T

    shared = {
        "hsT": np.ascontiguousarray(hs.T),
        "ehsT": np.ascontiguousarray(ehs.T),
        "maskp": np.ascontiguousarray(mask.reshape(TKT, 128).T),
        "ident": np.eye(128, dtype=np.float32),
    }
    onesP2 = np.zeros((128, 2), np.float32)
    onesP2[0:64, 0] = 1.0
    onesP2[64:128, 1] = 1.0
    shared["onesP2"] = onesP2
    shared["onesP1"] = np.ones((64, 1), np.float32)

    def blocks(w):
        a = np.zeros((2, 128), np.float32)
        a[0, 0:64] = w
        a[1, 64:128] = w
        return a, np.ascontiguousarray(w.reshape(1, 64))

    for nm, key in (("wq", "norm_q_weight"), ("wk", "norm_k_weight"),
                    ("wak", "norm_added_k_weight")):
        a, b = blocks(g[key].astype(np.float32))
        shared[nm + "bcA"] = a
        shared[nm + "bcB"] = b

    f32 = np.float32
    in_maps = []
    for c in range(NCORES):
        sl = slice(c * J, (c + 1) * J)
        wq = g["to_q_weight"].astype(f32)[sl, :].T    # [D, 192]
        wk = g["to_k_weight"].astype(f32)[sl, :].T
        wv = g["to_v_weight"].astype(f32)[sl, :].T
        awk = g["add_k_proj_weight"].astype(f32)[sl, :].T
        awv = g["add_v_proj_weight"].astype(f32)[sl, :].T
        m = dict(shared)
        m["wkA"] = np.ascontiguousarray(wk[:, 0:128])
        m["wqA"] = np.ascontiguousarray(wq[:, 0:128])
        m["wvA"] = np.ascontiguousarray(wv[:, 0:128])
        m["wqvB"] = np.ascontiguousarray(
            np.concatenate([wq[:, 128:192], wv[:, 128:192]], axis=1))
        m["wkB"] = np.ascontiguousarray(wk[:, 128:192])
        m["awkA"] = np.ascontiguousarray(awk[:, 0:128])
        m["awvA"] = np.ascontiguousarray(awv[:, 0:128])
        m["wkvBe"] = np.ascontiguousarray(
            np.concatenate([awk[:, 128:192], awv[:, 128:192]], axis=1))
        m["wo"] = np.ascontiguousarray(g["to_out_0_weight"].astype(f32)[:, sl].T)
        b5 = np.stack(
            [g["to_q_bias"].astype(f32)[sl],
             g["to_k_bias"].astype(f32)[sl],
             g["to_v_bias"].astype(f32)[sl],
             g["add_k_proj_bias"].astype(f32)[sl],
             g["add_v_proj_bias"].astype(f32)[sl]], axis=1)   # [192, 5]
        m["bias8"] = np.ascontiguousarray(
            np.concatenate([b5, b5[128:192, :]], axis=0))     # [256, 5]
        in_maps.append(m)
    return in_maps


def kernel(**inputs):
    global _cached_nc
    import time
    from concourse import bass_utils

    if _cached_nc is None:
        _cached_nc = _build()
    nc = _cached_nc

    in_maps = _prep_inputs(inputs)
    res = None
    last_err = None
    for attempt in range(3):
        try:
            res = bass_utils.run_bass_kernel_spmd(nc, in_maps,
                                                  core_ids=list(range(NCORES)))
            break
        except Exception as e:  # transient axon/device hiccups
            last_err = e
            time.sleep(5.0 * (attempt + 1))
    if res is None:
        raise last_err
    acc = res.results[0]["yT"].copy()
    for c in range(1, NCORES):
        acc += res.results[c]["yT"]
    bias = np.asarray(inputs["to_out_0_bias"]).astype(np.float32)
    out = acc.T + bias[None, :]
    return out.reshape(1, S, D).astype(np.float32)


if __name__ == "__main__":
    rng = np.random.default_rng(0)
    ins = {
        "hidden_states": rng.standard_normal((1, S, D)).astype(np.float32),
        "encoder_hidden_states": rng.standard_normal((1, SE, D)).astype(np.float32),
        "attention_mask": np.zeros((1, 1, 1, T), np.float32),
        "to_q_weight": (rng.standard_normal((D, D)) * 0.02).astype(np.float32),
        "to_q_bias": np.zeros(D, np.float32),
        "to_k_weight": (rng.standard_normal((D, D)) * 0.02).astype(np.float32),
        "to_k_bias": np.zeros(D, np.float32),
        "to_v_weight": (rng.standard_normal((D, D)) * 0.02).astype(np.float32),
        "to_v_bias": np.zeros(D, np.float32),
        "norm_q_weight": np.ones(HD, np.float32),
        "norm_k_weight": np.ones(HD, np.float32),
        "add_q_proj_weight": (rng.standard_normal((D, D)) * 0.02).astype(np.float32),
        "add_q_proj_bias": np.zeros(D, np.float32),
        "add_k_proj_weight": (rng.standard_normal((D, D)) * 0.02).astype(np.float32),
        "add_k_proj_bias": np.zeros(D, np.float32),
        "add_v_proj_weight": (rng.standard_normal((D, D)) * 0.02).astype(np.float32),
        "add_v_proj_bias": np.zeros(D, np.float32),
        "norm_added_q_weight": np.ones(HD, np.float32),
        "norm_added_k_weight": np.ones(HD, np.float32),
        "to_out_0_weight": (rng.standard_normal((D, D)) * 0.02).astype(np.float32),
        "to_out_0_bias": np.zeros(D, np.float32),
        "to_add_out_weight": (rng.standard_normal((D, D)) * 0.02).astype(np.float32),
        "to_add_out_bias": np.zeros(D, np.float32),
        "attn_heads": 24,
        "attn_head_dim": 64,
    }
    out = kernel(**ins)
    print("out", out.shape, out.dtype, float(np.abs(out).max()))
